# revision 3
# baseline (speedup 1.0000x reference)
"""Trainium2 Bass kernel for nn_BandSplitDCTFilter.

Math: the reference's mirror-FFT DCT / band filters / inverse collapse to
    out_c = C1 (Z_c) C2^T - S1 (Z_c) S2^T,   Z_c = (A x_c A^T) .* W_eff_c
with A[k,j] = 2cos(pi k (2j+1)/128), C1/S1 the cos/sin inverse bases,
C2/S2 carrying the irfft half-spectrum weights u_l (1 for l=0 else 2) and
the 1/(4HW) scale, and W_eff = pad(W_low)+pad(W_mid)+W_high merged into a
single spectral mask (all bands share the same inverse basis under
zero-padding). Then y = x_out @ proj_w^T followed by LayerNorm.

Sharding: pure data-parallel, one batch sample per NeuronCore (B=8 = 8
cores); the small transform/band/proj weights are replicated.

Per-core pipeline (partition layouts in brackets; chalf = c // 128,
cj = c % 128):
  S1  DMA-in xr [(chalf,h), (w,cj)]
  S2  F-h   : T1 = KH^T xr            [(chalf,k), (w,cj)]   (Kron blockdiag)
  S3  pivot : T2[(chalf,w), (k,cj)] = T1[(chalf,k), (w,cj)]  (64 SBUF DMAs)
  S4  F-w   : Z = (KH^T T2) .* Weff   [(chalf,l), (k,cj)]   (mask fused in drain)
  S5  I-l   : Uc = KC^T Z, Us = KS^T Z [(chalf,n), (k,cj)]  (bf16 drains)
  S6  pivot : Ustk[k | 64+k, (n,chalf,cj)] = Uc/Us           (128 SBUF DMAs)
  S7  I-k   : per (n,chalf): X[cj, m] = Ustk-chunk^T @ [C1^T; -S1^T]
  S8  proj  : per 128-row tile: Y = X0-chunk^T pjt0 + X1-chunk^T pjt1
  S9  LN    : bn_stats/bn_aggr + fused (y-mu)*rstd, -> DMA out
Host does layout-only prep/unprep (shard, rearrange, row unpermute).
"""

import numpy as np
import ml_dtypes

import bass_rust
import concourse.bass as bass
import concourse.mybir as mybir
from concourse.tile import TileContext, ScopedClock
from concourse.bass_utils import run_bass_kernel_spmd

# ---------------------------------------------------------------------------
# Workarounds for this container's walrus build: any instruction may carry at
# most ONE sync wait ("Too many sync wait commands" otherwise).
# ---------------------------------------------------------------------------

_wait_ctr = 0


def _split_multi_waits(nc, max_waits=1):
    global _wait_ctr
    for f in nc.m.functions:
        for bb in f.blocks:
            out = []
            dirty = False
            for ins in bb.instructions:
                si = ins.sync_info
                if si is not None and len(si.on_wait) > max_waits:
                    waits = list(si.on_wait)
                    for w in waits[:-max_waits]:
                        _wait_ctr += 1
                        nop = bass_rust.InstNoOp(name=f"I-waitsplit-{_wait_ctr}")
                        nop.engine = ins.engine
                        nop.sync_info = mybir.SyncInfo(on_wait=[w], on_update=[])
                        out.append(nop)
                    ins.sync_info = mybir.SyncInfo(
                        on_wait=waits[-max_waits:], on_update=list(si.on_update)
                    )
                    dirty = True
                out.append(ins)
            if dirty:
                bb.instructions = out


def _patched_drain_and_barrier(self, tick_clock, wait_clock):
    # Same single-wait limit applies to the Tile tail drain: spread the
    # outstanding waits over SP nops executed in order before the drain.
    nc = self.nc
    probe = nc.sync.nop(nofuse=True)
    wait_clock.add_sem_waits(probe.ins, ScopedClock({None: tick_clock.global_clock}))
    si = probe.ins.sync_info
    waits = list(si.on_wait) if si is not None else []
    probe.ins.sync_info = mybir.SyncInfo(on_wait=waits[:1], on_update=[])
    name2sem = {s.name: s for s in self.sems.allocated().values()}
    for w in waits[1:]:
        nc.sync.nop(nofuse=True)._wait_ge(name2sem[w.ant_name], w.wait_value)
    nc.sync.drain()
    nc.all_engine_barrier()
    popped = nc._tile_sem_poison_stack.pop()
    assert popped is self._sem_poison
    nc.clear_and_free_semaphores(list(self.sems.allocated().values()))
    nc.all_engine_barrier()


TileContext._drain_and_barrier = _patched_drain_and_barrier

# ---------------------------------------------------------------------------
# Problem constants (hardcoded per harness contract)
# ---------------------------------------------------------------------------

B, H, W, C = 8, 64, 64, 256
N = H * W
F32 = mybir.dt.float32
F32R = mybir.dt.float32r
BF16 = mybir.dt.bfloat16
ALU = mybir.AluOpType
ACTF = mybir.ActivationFunctionType


def _host_matrices():
    k = np.arange(64)
    j = np.arange(64)
    ang = np.pi * k[:, None] * (2 * j[None, :] + 1) / 128.0
    A = 2.0 * np.cos(ang)
    u = np.where(k == 0, 1.0, 2.0)
    C1T = np.cos(ang)                       # C1T[k,m] = C1[m,k]
    S1T = np.sin(ang)
    C2T = u[:, None] * np.cos(ang) / 16384.0
    S2T = u[:, None] * np.sin(ang) / 16384.0

    def blockdiag(M):
        out = np.zeros((128, 128), np.float32)
        out[:64, :64] = M
        out[64:, 64:] = M
        return out

    KH = blockdiag(A.T)
    KC = blockdiag(C2T)
    KS = blockdiag(S2T)
    ICS = np.concatenate([C1T, -S1T], axis=0).astype(ml_dtypes.bfloat16)
    return KH.astype(np.float32), KC.astype(np.float32), KS.astype(np.float32), ICS


# ---------------------------------------------------------------------------
# Device program
# ---------------------------------------------------------------------------

_NC_CACHE = {}


def _build_nc(apply_gb):
    nc = bass.Bass(trn_type="TRN2")

    xr_d = nc.dram_tensor("xr", [128, 8192], F32R, kind="ExternalInput")
    kh_d = nc.dram_tensor("kh", [128, 128], F32R, kind="ExternalInput")
    kc_d = nc.dram_tensor("kc", [128, 128], F32R, kind="ExternalInput")
    ks_d = nc.dram_tensor("ks", [128, 128], F32R, kind="ExternalInput")
    ics_d = nc.dram_tensor("ics", [128, 64], BF16, kind="ExternalInput")
    weff_d = nc.dram_tensor("weff", [128, 8192], F32, kind="ExternalInput")
    pjt_d = nc.dram_tensor("pjt", [128, 512], F32R, kind="ExternalInput")
    gb_d = nc.dram_tensor("gb", [2, 256], F32, kind="ExternalInput")
    y_d = nc.dram_tensor("y", [4096, 256], F32, kind="ExternalOutput")

    with TileContext(nc) as tc:
        with (
            tc.tile_pool(name="consts", bufs=1) as consts,
            tc.tile_pool(name="weffp", bufs=1) as weffp,
            tc.tile_pool(name="bigA", bufs=1) as bigA,
            tc.tile_pool(name="bigB", bufs=1) as bigB,
            tc.tile_pool(name="bigC", bufs=1) as bigC,
            tc.tile_pool(name="zp", bufs=1) as zp,
            tc.tile_pool(name="ps", bufs=8, space="PSUM") as ps,
            tc.tile_pool(name="small", bufs=8) as small,
            tc.tile_pool(name="yout", bufs=4) as yout,
        ):
            # ---- constants in ----
            kh = consts.tile([128, 128], F32R, tag="kh")
            kc = consts.tile([128, 128], F32R, tag="kc")
            ks = consts.tile([128, 128], F32R, tag="ks")
            ics = consts.tile([128, 64], BF16, tag="ics")
            pjt = consts.tile([128, 512], F32R, tag="pjt")
            weff = weffp.tile([128, 8192], F32, tag="weff")
            nc.sync.dma_start(out=kh[:], in_=kh_d[:])
            nc.sync.dma_start(out=kc[:], in_=kc_d[:])
            nc.sync.dma_start(out=ks[:], in_=ks_d[:])
            nc.sync.dma_start(out=ics[:], in_=ics_d[:])
            nc.sync.dma_start(out=pjt[:], in_=pjt_d[:])
            nc.sync.dma_start(out=weff[:], in_=weff_d[:])
            eps = consts.tile([128, 1], F32, tag="eps")
            nc.vector.memset(eps[:], 1e-5)
            if apply_gb:
                gt = consts.tile([128, 256], F32, tag="gt")
                bt = consts.tile([128, 256], F32, tag="bt")
                gb_ap = gb_d.ap()
                g_b = bass.AP(tensor=gb_ap.tensor, offset=0, ap=[[0, 128], [1, 256]])
                b_b = bass.AP(tensor=gb_ap.tensor, offset=256, ap=[[0, 128], [1, 256]])
                nc.sync.dma_start(out=gt[:], in_=g_b)
                nc.sync.dma_start(out=bt[:], in_=b_b)

            # ---- S1: x in ----
            X = bigA.tile([128, 8192], F32R, tag="bigA")
            nc.sync.dma_start(out=X[:], in_=xr_d[:])

            # ---- S2: F-h ----
            T1 = bigB.tile([128, 8192], F32R, tag="bigB")
            for i in range(16):
                pt = ps.tile([128, 512], F32, tag="ps")
                nc.tensor.matmul(
                    pt[:], kh[:], X[:, i * 512:(i + 1) * 512], start=True, stop=True
                )
                eng = nc.vector.tensor_copy if i % 2 == 0 else nc.scalar.copy
                eng(T1[:, i * 512:(i + 1) * 512], pt[:])

            # ---- S3: pivot-1 (64 SBUF->SBUF DMAs) ----
            T2 = bigC.tile([128, 8192], F32R, tag="bigC")
            T2v = T2[:].rearrange("(c2 w) f -> c2 w f", c2=2)
            for w in range(64):
                nc.sync.dma_start(
                    out=T2v[:, w, :], in_=T1[:, w * 128:(w + 1) * 128]
                )

            # ---- S4: F-w, mask fused into drain ----
            Z = zp.tile([128, 8192], F32R, tag="z")
            for i in range(16):
                pt = ps.tile([128, 512], F32, tag="ps")
                nc.tensor.matmul(
                    pt[:], kh[:], T2[:, i * 512:(i + 1) * 512], start=True, stop=True
                )
                nc.vector.tensor_mul(
                    Z[:, i * 512:(i + 1) * 512], pt[:], weff[:, i * 512:(i + 1) * 512]
                )

            # ---- S5: I-l (cos+sin), bf16 drains ----
            U2 = bigB.tile([128, 16384], BF16, tag="bigB")
            for i in range(16):
                ptc = ps.tile([128, 512], F32, tag="ps")
                nc.tensor.matmul(
                    ptc[:], kc[:], Z[:, i * 512:(i + 1) * 512], start=True, stop=True
                )
                nc.scalar.copy(U2[:, i * 512:(i + 1) * 512], ptc[:])
                pts = ps.tile([128, 512], F32, tag="ps")
                nc.tensor.matmul(
                    pts[:], ks[:], Z[:, i * 512:(i + 1) * 512], start=True, stop=True
                )
                nc.vector.tensor_copy(U2[:, 8192 + i * 512:8192 + (i + 1) * 512], pts[:])

            # ---- S6: pivot-2 (128 SBUF->SBUF DMAs into k-stacked layout) ----
            Ustk = bigA.tile([128, 16384], BF16, tag="bigA")
            for k in range(64):
                for row, base in ((k, 0), (64 + k, 8192)):
                    dstv = Ustk[row:row + 1, :].rearrange(
                        "p (n c2 cj) -> p c2 n cj", c2=2, cj=128
                    )
                    for chalf in range(2):
                        nc.sync.dma_start(
                            out=dstv[:, chalf, :, :],
                            in_=U2[
                                chalf * 64:(chalf + 1) * 64,
                                base + k * 128:base + (k + 1) * 128,
                            ],
                        )

            # ---- S7: I-k, (b)-mode, grouped into [128,512] psum tiles ----
            X01 = bigC.tile([128, 8192], F32R, tag="bigC")
            X01v = X01[:].rearrange("p (c2 n m) -> p c2 n m", c2=2, m=64)
            for g in range(16):
                pt = ps.tile([128, 512], F32, tag="ps")
                for chalf in range(2):
                    for nn in range(4):
                        slot = chalf * 4 + nn
                        t = 8 * g + nn * 2 + chalf
                        nc.tensor.matmul(
                            pt[:, slot * 64:(slot + 1) * 64],
                            Ustk[:, t * 128:(t + 1) * 128],
                            ics[:],
                            start=True,
                            stop=True,
                        )
                eng = nc.vector.tensor_copy if g % 2 == 0 else nc.scalar.copy
                eng(X01v[:, :, 4 * g:4 * (g + 1), :], pt[:])

            # ---- S8 + S9: proj, LayerNorm, out ----
            for t2 in range(32):
                pty = ps.tile([128, 256], F32, tag="ps")
                nc.tensor.matmul(
                    pty[:], X01[:, t2 * 128:(t2 + 1) * 128], pjt[:, 0:256],
                    start=True, stop=False,
                )
                nc.tensor.matmul(
                    pty[:], X01[:, 4096 + t2 * 128:4096 + (t2 + 1) * 128],
                    pjt[:, 256:512],
                    start=False, stop=True,
                )
                stats = small.tile([128, 6], F32, tag="stats")
                mv = small.tile([128, 2], F32, tag="mv")
                nc.vector.bn_stats(out=stats[:], in_=pty[:])
                nc.vector.bn_aggr(out=mv[:], in_=stats[:])
                negmu = small.tile([128, 1], F32, tag="negmu")
                rstd = small.tile([128, 1], F32, tag="rstd")
                nc.vector.tensor_scalar_mul(negmu[:], mv[:, 0:1], -1.0)
                nc.scalar.activation(
                    out=rstd[:], in_=mv[:, 1:2], func=ACTF.Sqrt,
                    bias=eps[:], scale=1.0,
                )
                nc.vector.reciprocal(rstd[:], rstd[:])
                yt = yout.tile([128, 256], F32, tag="yt")
                nc.vector.tensor_scalar(
                    yt[:], pty[:], negmu[:], rstd[:], op0=ALU.add, op1=ALU.mult
                )
                if apply_gb:
                    nc.vector.tensor_mul(yt[:], yt[:], gt[:])
                    nc.vector.tensor_add(yt[:], yt[:], bt[:])
                nc.sync.dma_start(
                    out=y_d[t2 * 128:(t2 + 1) * 128, :], in_=yt[:]
                )

    _split_multi_waits(nc)
    return nc


def _get_nc(apply_gb):
    key = bool(apply_gb)
    if key not in _NC_CACHE:
        _NC_CACHE[key] = _build_nc(key)
    return _NC_CACHE[key]


# ---------------------------------------------------------------------------
# Host entry point
# ---------------------------------------------------------------------------

def kernel(x, W_low, W_mid, W_high, proj_w, ln_g, ln_b):
    x = np.ascontiguousarray(np.asarray(x, dtype=np.float32))
    W_low = np.asarray(W_low, dtype=np.float32)
    W_mid = np.asarray(W_mid, dtype=np.float32)
    W_high = np.asarray(W_high, dtype=np.float32)
    proj_w = np.asarray(proj_w, dtype=np.float32)
    ln_g = np.asarray(ln_g, dtype=np.float32)
    ln_b = np.asarray(ln_b, dtype=np.float32)

    KH, KC, KS, ICS = _host_matrices()

    W_eff = W_high[0].copy()
    W_eff[:32, :32] += W_mid[0]
    W_eff[:16, :16] += W_low[0]
    weff_r = np.ascontiguousarray(
        W_eff.reshape(64, 64, 2, 128).transpose(2, 1, 0, 3).reshape(128, 8192)
    )

    pjt = np.zeros((128, 512), np.float32)
    pjt[:, :256] = proj_w.T[:128]
    pjt[:, 256:] = proj_w.T[128:]

    apply_gb = not (np.all(ln_g == 1.0) and np.all(ln_b == 0.0))
    gb = np.stack([ln_g, ln_b]).astype(np.float32)

    consts = {
        "kh": KH, "kc": KC, "ks": KS,
        "ics": np.ascontiguousarray(ICS),
        "weff": weff_r, "pjt": pjt, "gb": gb,
    }

    in_maps = []
    for b in range(B):
        xr = np.ascontiguousarray(
            x[b].reshape(64, 64, 2, 128).transpose(2, 0, 1, 3).reshape(128, 8192)
        )
        m = dict(consts)
        m["xr"] = xr
        in_maps.append(m)

    nc = _get_nc(apply_gb)
    res = run_bass_kernel_spmd(nc, in_maps, core_ids=list(range(B)))

    out = np.empty((B, N, C), np.float32)
    for b in range(B):
        yc = res.results[b]["y"]
        out[b] = yc.reshape(64, 64, 256).transpose(1, 0, 2).reshape(4096, 256)
    return out


# revision 7
# speedup vs baseline: 1.7313x; 1.7313x over previous
"""Trainium2 Bass kernel for nn_BandSplitDCTFilter.

Math: the reference's mirror-FFT DCT / band filter / inverse collapses to
    out_c = C1 (Z_c) C2^T - S1 (Z_c) S2^T,   Z_c = (A x_c A^T) .* W_eff_c
with A[k,j] = 2cos(pi k (2j+1)/128); C2/S2 carry the irfft half-spectrum
weights u_l and the 1/(4HW) scale; W_eff = pad(W_low)+pad(W_mid)+W_high
merges the three bands (they share the inverse basis under zero-padding).
Then y = x_out @ proj_w^T and LayerNorm.

Sharding: pure data-parallel, one sample per core (B=8 = 8 cores), small
weights replicated.

Per-core pipeline (v9). 64-row tensors are "packed": free dim split in
half across partition ranges [0:64) and [64:128) so every engine op and
PSUM tile runs 128 partitions wide. Layout pivots ride DRAM (strided
store with >=512B runs + contiguous reload) to keep the DMA instruction
count tiny — this container pays ~0.6-0.8us of sequencer time per DMA
instruction, so many small DMAs serialize.

  S2  F-h : T1[k,(w,c)]   = AT.T @ x[h,(w,c)]      (2 packed halves)
  P1  T1 -> DRAM (k,w,c order) -> T2[w,(k,c)]       (2+2 DMA instrs)
  S4  F-w : Z[l,(k,c)]    = AT.T @ T2, * W_eff fused in drain
  S5  I-l : U2s[(cs,n),(k,c)] = [C2T|S2T].T @ Z     (cos/sin stacked)
  P2  U2s -> DRAM ((cs,k),(n,c) order) -> Ustk      (2+1 DMA instrs, bf16)
  S7  I-k : per (n,chalf): X01[cj, m] = Ustk-chunk.T @ [C1T;-S1T]
  S8  proj: per 128-row tile: Y = X0c.T pjt0 + X1c.T pjt1
  S9  LN  : bn_stats/aggr + fused (y-mu)*rstd -> Yall -> 1 DMA out
Host does layout-only prep (shard/pack) and row unpermute.
"""

import numpy as np
import ml_dtypes

import bass_rust
import concourse.bass as bass
import concourse.mybir as mybir
from concourse.tile import TileContext, ScopedClock
from concourse.bass_utils import run_bass_kernel_spmd

# ---------------------------------------------------------------------------
# Workarounds: this container's walrus rejects >1 sync wait per instruction.
# ---------------------------------------------------------------------------

_wait_ctr = 0


def _split_multi_waits(nc, max_waits=1):
    global _wait_ctr
    for f in nc.m.functions:
        for bb in f.blocks:
            out = []
            dirty = False
            for ins in bb.instructions:
                si = ins.sync_info
                if si is not None and len(si.on_wait) > max_waits:
                    waits = list(si.on_wait)
                    for w in waits[:-max_waits]:
                        _wait_ctr += 1
                        nop = bass_rust.InstNoOp(name=f"I-waitsplit-{_wait_ctr}")
                        nop.engine = ins.engine
                        nop.sync_info = mybir.SyncInfo(on_wait=[w], on_update=[])
                        out.append(nop)
                    ins.sync_info = mybir.SyncInfo(
                        on_wait=waits[-max_waits:], on_update=list(si.on_update)
                    )
                    dirty = True
                out.append(ins)
            if dirty:
                bb.instructions = out


def _patched_drain_and_barrier(self, tick_clock, wait_clock):
    nc = self.nc
    probe = nc.sync.nop(nofuse=True)
    wait_clock.add_sem_waits(probe.ins, ScopedClock({None: tick_clock.global_clock}))
    si = probe.ins.sync_info
    waits = list(si.on_wait) if si is not None else []
    probe.ins.sync_info = mybir.SyncInfo(on_wait=waits[:1], on_update=[])
    name2sem = {s.name: s for s in self.sems.allocated().values()}
    for w in waits[1:]:
        nc.sync.nop(nofuse=True)._wait_ge(name2sem[w.ant_name], w.wait_value)
    nc.sync.drain()
    nc.all_engine_barrier()
    popped = nc._tile_sem_poison_stack.pop()
    assert popped is self._sem_poison
    nc.clear_and_free_semaphores(list(self.sems.allocated().values()))
    nc.all_engine_barrier()


TileContext._drain_and_barrier = _patched_drain_and_barrier

# ---------------------------------------------------------------------------

B, H, W, C = 8, 64, 64, 256
N = H * W
F32 = mybir.dt.float32
F32R = mybir.dt.float32r
BF16 = mybir.dt.bfloat16
ALU = mybir.AluOpType
ACTF = mybir.ActivationFunctionType


def _host_matrices():
    k = np.arange(64)
    j = np.arange(64)
    ang = np.pi * k[:, None] * (2 * j[None, :] + 1) / 128.0
    A = 2.0 * np.cos(ang)
    u = np.where(k == 0, 1.0, 2.0)
    C1T = np.cos(ang)
    S1T = np.sin(ang)
    C2T = u[:, None] * np.cos(ang) / 16384.0
    S2T = u[:, None] * np.sin(ang) / 16384.0

    AT = A.T.astype(np.float32)                                   # [h, k]
    kh2 = np.concatenate([AT, AT], axis=0).astype(np.float32)     # [128, 64]
    cs2_half = np.concatenate([C2T, S2T], axis=1)                 # [l, 128]
    cs2 = np.concatenate([cs2_half, cs2_half], axis=0).astype(np.float32)
    ICS = np.concatenate([C1T, -S1T], axis=0).astype(ml_dtypes.bfloat16)
    return kh2, cs2, np.ascontiguousarray(ICS)


_NC_CACHE = {}


def _build_nc(apply_gb):
    nc = bass.Bass(trn_type="TRN2")

    xr_d = nc.dram_tensor("xr", [128, 8192], F32R, kind="ExternalInput")
    kh_d = nc.dram_tensor("kh", [128, 64], F32R, kind="ExternalInput")
    cs_d = nc.dram_tensor("cs", [128, 128], F32R, kind="ExternalInput")
    ics_d = nc.dram_tensor("ics", [128, 64], BF16, kind="ExternalInput")
    weff_d = nc.dram_tensor("weff", [128, 8192], F32, kind="ExternalInput")
    pjt_d = nc.dram_tensor("pjt", [128, 512], BF16, kind="ExternalInput")
    gb_d = nc.dram_tensor("gb", [2, 256], F32, kind="ExternalInput")
    y_d = nc.dram_tensor("y", [4096, 256], F32, kind="ExternalOutput")

    with TileContext(nc) as tc:
        with (
            tc.tile_pool(name="consts", bufs=1) as consts,
            tc.tile_pool(name="weffp", bufs=1) as weffp,
            tc.tile_pool(name="cA", bufs=1) as cA,
            tc.tile_pool(name="cB", bufs=1) as cB,
            tc.tile_pool(name="cC", bufs=1) as cC,
            tc.tile_pool(name="cD", bufs=1) as cD,
            tc.tile_pool(name="u2p", bufs=1) as u2p,
            tc.tile_pool(name="dramp", bufs=1, space="DRAM") as dramp,
            tc.tile_pool(name="ps", bufs=8, space="PSUM") as ps,
            tc.tile_pool(name="small", bufs=8) as small,
        ):
            # ---- constants ----
            kh2 = consts.tile([128, 64], F32R, tag="kh2")
            cs2 = consts.tile([128, 128], F32R, tag="cs2")
            ics = consts.tile([128, 64], BF16, tag="ics")
            pjt = consts.tile([128, 512], BF16, tag="pjt")
            weff = weffp.tile([128, 8192], F32, tag="weff")
            nc.sync.dma_start(out=kh2[:], in_=kh_d[:])
            nc.sync.dma_start(out=cs2[:], in_=cs_d[:])
            nc.sync.dma_start(out=ics[:], in_=ics_d[:])
            nc.sync.dma_start(out=pjt[:], in_=pjt_d[:])
            nc.scalar.dma_start(out=weff[:], in_=weff_d[:])
            eps = consts.tile([128, 1], F32, tag="eps")
            nc.vector.memset(eps[:], 1e-5)
            if apply_gb:
                gt = consts.tile([128, 256], F32, tag="gt")
                bt = consts.tile([128, 256], F32, tag="bt")
                gb_ap = gb_d.ap()
                g_b = bass.AP(tensor=gb_ap.tensor, offset=0, ap=[[0, 128], [1, 256]])
                b_b = bass.AP(tensor=gb_ap.tensor, offset=256, ap=[[0, 128], [1, 256]])
                nc.sync.dma_start(out=gt[:], in_=g_b)
                nc.sync.dma_start(out=bt[:], in_=b_b)

            # ---- S1: x in (packed [128, 8192]) ----
            X = cA.tile([128, 8192], F32R, tag="cA")
            nc.sync.dma_start(out=X[:], in_=xr_d[:])

            # ---- S2: F-h ----
            T1p = cB.tile([128, 8192], F32R, tag="cB")
            for j in range(32):
                off = 64 * (j // 16)
                sl = slice((j % 16) * 512, (j % 16 + 1) * 512)
                pt = ps.tile([64, 512], F32, tag="ps")
                nc.tensor.matmul(pt[:], kh2[off:off + 64, :], X[off:off + 64, sl],
                                 start=True, stop=True)
                eng = nc.vector.tensor_copy if j % 2 == 0 else nc.scalar.copy
                eng(T1p[off:off + 64, sl], pt[:])

            # ---- P1: pivot via DRAM (store (k,w,c)-ordered, reload contig) ----
            D1 = dramp.tile([64, 16384], F32R, tag="d1")
            D1v = D1[:].rearrange("w (k c) -> k w c", c=256)
            nc.sync.dma_start(out=D1v[:, 0:32, :], in_=T1p[0:64, :])
            nc.scalar.dma_start(out=D1v[:, 32:64, :], in_=T1p[64:128, :])
            T2p = cC.tile([128, 8192], F32R, tag="cC")
            nc.sync.dma_start(out=T2p[0:64, :], in_=D1[:, 0:8192])
            nc.scalar.dma_start(out=T2p[64:128, :], in_=D1[:, 8192:16384])

            # ---- S4: F-w + mask fused in drain ----
            Zp = cD.tile([128, 8192], F32R, tag="cD")
            for j in range(32):
                off = 64 * (j // 16)
                sl = slice((j % 16) * 512, (j % 16 + 1) * 512)
                pt = ps.tile([64, 512], F32, tag="ps")
                nc.tensor.matmul(pt[:], kh2[off:off + 64, :], T2p[off:off + 64, sl],
                                 start=True, stop=True)
                nc.vector.tensor_mul(Zp[off:off + 64, sl], pt[:],
                                     weff[off:off + 64, sl])

            # ---- S5: I-l, cos/sin stacked into M=128 ----
            U2s = u2p.tile([128, 16384], BF16, tag="u2s")
            for j in range(32):
                half = j // 16
                off = 64 * half
                sl = slice((j % 16) * 512, (j % 16 + 1) * 512)
                pt = ps.tile([128, 512], F32, tag="ps")
                nc.tensor.matmul(pt[:], cs2[off:off + 64, :], Zp[off:off + 64, sl],
                                 start=True, stop=True)
                dsl = slice(j * 512, (j + 1) * 512)
                eng = nc.vector.tensor_copy if j % 2 == 0 else nc.scalar.copy
                eng(U2s[:, dsl], pt[:])

            # ---- P2: pivot via DRAM (bf16) ----
            D2 = dramp.tile([128, 16384], BF16, tag="d2")
            for cshalf in range(2):
                dst = D2[cshalf * 64:(cshalf + 1) * 64, :].rearrange(
                    "k (n c) -> n k c", c=256
                )
                eng = nc.sync if cshalf == 0 else nc.scalar
                eng.dma_start(out=dst, in_=U2s[cshalf * 64:(cshalf + 1) * 64, :])
            Ustk = cA.tile([128, 16384], BF16, tag="cA")
            nc.sync.dma_start(out=Ustk[:], in_=D2[:])

            # ---- S7: I-k, (b)-mode, 8 chunks per psum tile ----
            X01 = cC.tile([128, 8192], BF16, tag="cC")
            X01v = X01[:].rearrange("p (c2 n m) -> p c2 n m", c2=2, m=64)
            for g in range(16):
                pt = ps.tile([128, 512], F32, tag="ps")
                for chalf in range(2):
                    for nn in range(4):
                        slot = chalf * 4 + nn
                        t = 8 * g + nn * 2 + chalf
                        nc.tensor.matmul(
                            pt[:, slot * 64:(slot + 1) * 64],
                            Ustk[:, t * 128:(t + 1) * 128],
                            ics[:],
                            start=True, stop=True,
                        )
                eng = nc.vector.tensor_copy if g % 2 == 0 else nc.scalar.copy
                eng(X01v[:, :, 4 * g:4 * (g + 1), :], pt[:])

            # ---- S8 + S9: proj, LayerNorm -> Yall ----
            Yall = cD.tile([128, 8192], F32, tag="cD")
            for t2 in range(32):
                pty = ps.tile([128, 256], F32, tag="ps")
                nc.tensor.matmul(pty[:], X01[:, t2 * 128:(t2 + 1) * 128],
                                 pjt[:, 0:256], start=True, stop=False)
                nc.tensor.matmul(pty[:], X01[:, 4096 + t2 * 128:4096 + (t2 + 1) * 128],
                                 pjt[:, 256:512], start=False, stop=True)
                stats = small.tile([128, 6], F32, tag="stats")
                mv = small.tile([128, 2], F32, tag="mv")
                nc.vector.bn_stats(out=stats[:], in_=pty[:])
                nc.vector.bn_aggr(out=mv[:], in_=stats[:])
                rstd = small.tile([128, 1], F32, tag="rstd")
                negmu = small.tile([128, 1], F32, tag="negmu")
                nc.scalar.activation(out=rstd[:], in_=mv[:, 1:2], func=ACTF.Sqrt,
                                     bias=eps[:], scale=1.0)
                nc.vector.reciprocal(rstd[:], rstd[:])
                nc.vector.tensor_scalar_mul(negmu[:], mv[:, 0:1], -1.0)
                ysl = slice(t2 * 256, (t2 + 1) * 256)
                nc.vector.tensor_scalar(Yall[:, ysl], pty[:], negmu[:], rstd[:],
                                        op0=ALU.add, op1=ALU.mult)
                if apply_gb:
                    nc.vector.tensor_mul(Yall[:, ysl], Yall[:, ysl], gt[:])
                    nc.vector.tensor_add(Yall[:, ysl], Yall[:, ysl], bt[:])

            # ---- S10: one strided store ----
            yv = y_d[:].rearrange("(t r) d -> r t d", r=128)
            nc.scalar.dma_start(out=yv, in_=Yall[:])

    _split_multi_waits(nc)
    return nc


def _get_nc(apply_gb):
    key = bool(apply_gb)
    if key not in _NC_CACHE:
        _NC_CACHE[key] = _build_nc(key)
    return _NC_CACHE[key]


def _make_inputs(x, W_low, W_mid, W_high, proj_w, ln_g, ln_b):
    kh2, cs2, ICS = _host_matrices()

    W_eff = W_high[0].copy()
    W_eff[:32, :32] += W_mid[0]
    W_eff[:16, :16] += W_low[0]
    weff_r = W_eff.transpose(1, 0, 2).reshape(64, 16384)
    weff_p = np.ascontiguousarray(
        weff_r.reshape(64, 2, 8192).transpose(1, 0, 2).reshape(128, 8192)
    )

    pjt = np.zeros((128, 512), ml_dtypes.bfloat16)
    pjt[:, :256] = proj_w.T[:128]
    pjt[:, 256:] = proj_w.T[128:]

    gb = np.stack([ln_g, ln_b]).astype(np.float32)
    consts = {"kh": kh2, "cs": cs2, "ics": ICS,
              "weff": weff_p, "pjt": pjt, "gb": gb}

    in_maps = []
    for b in range(B):
        xr = np.ascontiguousarray(
            x[b].reshape(64, 2, 32, 256).transpose(1, 0, 2, 3).reshape(128, 8192)
        )
        m = dict(consts)
        m["xr"] = xr
        in_maps.append(m)
    return in_maps


def kernel(x, W_low, W_mid, W_high, proj_w, ln_g, ln_b):
    x = np.ascontiguousarray(np.asarray(x, dtype=np.float32))
    W_low = np.asarray(W_low, dtype=np.float32)
    W_mid = np.asarray(W_mid, dtype=np.float32)
    W_high = np.asarray(W_high, dtype=np.float32)
    proj_w = np.asarray(proj_w, dtype=np.float32)
    ln_g = np.asarray(ln_g, dtype=np.float32)
    ln_b = np.asarray(ln_b, dtype=np.float32)

    apply_gb = not (np.all(ln_g == 1.0) and np.all(ln_b == 0.0))
    in_maps = _make_inputs(x, W_low, W_mid, W_high, proj_w, ln_g, ln_b)
    nc = _get_nc(apply_gb)
    res = run_bass_kernel_spmd(nc, in_maps, core_ids=list(range(B)))

    out = np.empty((B, N, C), np.float32)
    for b in range(B):
        yc = res.results[b]["y"]
        out[b] = yc.reshape(64, 64, 256).transpose(1, 0, 2).reshape(4096, 256)
    return out
